# revision 41
# baseline (speedup 1.0000x reference)
"""GCN (3-layer DGL GraphConv, norm='both') on 8 TRN2 NeuronCores.

Strategy: nodes partitioned across cores by dst range. Per layer:
  - per-edge messages gathered via indirect DMA from a replicated node table
  - segment-sum over dst implemented as a triangular-matmul cumsum along
    partitions + two boundary gathers + subtract (no scatter needed; edges
    are host-packed so no node's run crosses a 128-slot column)
  - tiny dense GEMMs (W0 outer-product / W1 / W2) on PE
  - AllGather replicates each core's message-table slice between layers.
Host does index-only preprocessing (sort/pack/degree counts); all float
math runs on device.
"""

import sys

import numpy as np

try:
    import concourse.bass as bass  # noqa: F401
except Exception:  # pragma: no cover
    sys.path.insert(0, "/opt/trn_rl_repo")

import concourse.bass as bass
import concourse.bacc as bacc
import concourse.tile as tile
from concourse import mybir
from concourse.bass_utils import run_bass_kernel_spmd
from concourse.masks import make_upper_triangular

N_CORES = 8
N_NODES = 50000
NPC = N_NODES // N_CORES  # 6250 nodes per core
NPAD = 6272               # 49 * 128
NI = NPAD // 128          # 49
TBL = N_CORES * NPAD      # 50176 table rows
TW = TBL // 128           # 392
F1 = 100
F2 = 10
P = 128
GHOST_G = 6250                       # core-0 pad row, G layout (zeroed)
GHOST_K = (6250 % 128) * NI + 6250 // 128  # 5242, core-0 pad row, K layout
DT = mybir.dt.float32
IT = mybir.dt.int32


def _kappa(m):
    return (m % 128) * NI + m // 128


def _preprocess(edge_index):
    """Index-only host prep. Returns per-core dict arrays + W."""
    src = edge_index[0].astype(np.int64)
    dst = edge_index[1].astype(np.int64)
    deg_src = np.bincount(src, minlength=N_NODES).astype(np.float32)
    deg_dst = np.bincount(dst, minlength=N_NODES).astype(np.float32)
    order = np.argsort(dst, kind="stable")
    src_s = src[order]
    dst_s = dst[order]
    bounds = np.searchsorted(dst_s, np.arange(0, N_NODES + 1, NPC))

    packs = []
    Wmax = 0
    for c in range(N_CORES):
        lo, hi = bounds[c], bounds[c + 1]
        s_c = src_s[lo:hi]
        d_c = (dst_s[lo:hi] - c * NPC).astype(np.int64)
        lens = np.bincount(d_c, minlength=NPC).astype(np.int64)
        assert lens.max() <= 128, f"node degree {lens.max()} exceeds 128"
        # first-fit-decreasing bin-packing of node runs into 128-slot columns
        # (cuts columns ~5% vs node-order greedy; fewer gather instructions)
        col = np.zeros(NPC, np.int64)
        slot0 = np.zeros(NPC, np.int64)
        fillarr = np.zeros(1024, np.int64)
        ncols = 0
        for n in np.argsort(-lens, kind="stable"):
            L = lens[n]
            if L == 0:
                break
            j = int(np.argmax(fillarr[:ncols] + L <= 128)) if ncols else 0
            if ncols == 0 or fillarr[j] + L > 128:
                j = ncols
                ncols += 1
            col[n] = j
            slot0[n] = fillarr[j]
            fillarr[j] += L
        Wc = ncols
        Wmax = max(Wmax, Wc)
        packs.append((s_c, d_c, lens, col, slot0))

    W = Wmax
    ZERO_POS = 128 * W
    cores = []
    for c in range(N_CORES):
        s_c, d_c, lens, col, slot0 = packs[c]
        run_start = np.zeros(NPC + 1, np.int64)
        np.cumsum(lens, out=run_start[1:])
        e_n = d_c
        j_e = np.arange(len(s_c)) - run_start[e_n]
        p_e = slot0[e_n] + j_e
        i_e = col[e_n]
        sc_core = s_c // NPC
        sc_loc = s_c % NPC
        sidx_g = np.full((128, W), GHOST_G, np.int32)
        sidx_k = np.full((128, W), GHOST_K, np.int32)
        sidx_g[p_e, i_e] = (sc_core * NPAD + sc_loc).astype(np.int32)
        sidx_k[p_e, i_e] = (sc_core * NPAD + _kappa(sc_loc)).astype(np.int32)

        bnd_end = np.full(NPAD, ZERO_POS, np.int64)
        bnd_beg = np.full(NPAD, ZERO_POS, np.int64)
        has = lens > 0
        idxs = np.nonzero(has)[0]
        bnd_end[idxs] = (slot0[idxs] + lens[idxs] - 1) * W + col[idxs]
        bnd_beg[idxs] = np.where(
            slot0[idxs] > 0, (slot0[idxs] - 1) * W + col[idxs], ZERO_POS
        )

        dd_own = np.ones(NPAD, np.float32)
        dd_own[:NPC] = deg_dst[c * NPC : (c + 1) * NPC]
        # pad rows get a huge degree so inv_sqrt ~ 0 zeroes their messages
        # (they are the target of ghost-edge gathers in the next layer)
        dsg = np.full(NPAD, 1e30, np.float32)
        dsg[:NPC] = deg_src[c * NPC : (c + 1) * NPC]
        dsk = np.full(NPAD, 1e30, np.float32)
        dsk[_kappa(np.arange(NPC))] = deg_src[c * NPC : (c + 1) * NPC]
        cores.append(
            dict(
                sidx_g=sidx_g,
                sidx_k=sidx_k,
                bnd_end=bnd_end.astype(np.int32).reshape(128, NI),
                bnd_beg=bnd_beg.astype(np.int32).reshape(128, NI),
                deg_dst_own=dd_own,
                deg_src_g=dsg,
                deg_src_k=dsk,
            )
        )

    # table-space (G layout) full arrays, same for every core
    deg_src_t = np.ones(TBL, np.float32)
    for c in range(N_CORES):
        deg_src_t[c * NPAD : c * NPAD + NPC] = deg_src[c * NPC : (c + 1) * NPC]
    return cores, deg_src_t, W


def _rsqrt(nc, pool, out, in_ap, tmp_tag):
    """out = 1/sqrt(max(in,1)) elementwise."""
    t1 = pool.tile(list(out.shape), DT, name=f"rs1_{tmp_tag}", tag=f"rs1_{tmp_tag}")
    nc.vector.tensor_scalar_max(t1[:], in_ap, 1.0)
    t2 = pool.tile(list(out.shape), DT, name=f"rs2_{tmp_tag}", tag=f"rs2_{tmp_tag}")
    nc.scalar.sqrt(t2[:], t1[:])
    nc.vector.reciprocal(out, t2[:])


def _build(W, dbg=False, timing=False):
    """Build the SPMD Bass program (identical for all cores).

    timing=True replaces AllGathers with local DRAM copies so the module is
    collective-free and TimelineSim-compatible (numerics wrong, timing of
    local work representative)."""
    ZROW = 128 * W
    nc = bacc.Bacc(
        "TRN2",
        target_bir_lowering=False,
        debug=False,
        num_devices=1 if timing else N_CORES,
    )
    dbg_specs = {
        "dbg_m0": [128, W],
        "dbg_cs0": [128, W],
        "dbg_agg0": [128, NI],
        "dbg_t0row": [1, NPAD],
        "dbg_t1sl": [128, NI * F2],
        "dbg_T1": [TBL, F2],
        "dbg_m1": [128, W * F2],
        "dbg_agg1": [128, NI * F2],
        "dbg_t2sl": [128, NI],
        "dbg_T2": [TBL, 1],
        "dbg_m2": [128, W],
        "dbg_agg2": [128, NI],
    }
    dbg_t = {}
    if dbg:
        for name, shp in dbg_specs.items():
            dbg_t[name] = nc.dram_tensor(name, shp, DT, kind="ExternalOutput")

    def tap(name, ap):
        if dbg:
            nc.sync.dma_start(dbg_t[name].ap(), ap)
    # ---- I/O -----------------------------------------------------------
    x_t = nc.dram_tensor("x_t", [TBL], DT, kind="ExternalInput")
    deg_src_t = nc.dram_tensor("deg_src_t", [TBL], DT, kind="ExternalInput")
    deg_dst_own = nc.dram_tensor("deg_dst_own", [NPAD], DT, kind="ExternalInput")
    deg_src_g = nc.dram_tensor("deg_src_g", [NPAD], DT, kind="ExternalInput")
    deg_src_k = nc.dram_tensor("deg_src_k", [NPAD], DT, kind="ExternalInput")
    sidx_g = nc.dram_tensor("sidx_g", [128, W], IT, kind="ExternalInput")
    sidx_k = nc.dram_tensor("sidx_k", [128, W], IT, kind="ExternalInput")
    bnd_beg = nc.dram_tensor("bnd_beg", [128, NI], IT, kind="ExternalInput")
    bnd_end = nc.dram_tensor("bnd_end", [128, NI], IT, kind="ExternalInput")
    w0 = nc.dram_tensor("w0", [1, F1], DT, kind="ExternalInput")
    b0 = nc.dram_tensor("b0", [F1], DT, kind="ExternalInput")
    w1 = nc.dram_tensor("w1", [F1, F2], DT, kind="ExternalInput")
    b1 = nc.dram_tensor("b1", [F2], DT, kind="ExternalInput")
    w2 = nc.dram_tensor("w2", [F2], DT, kind="ExternalInput")
    b2 = nc.dram_tensor("b2", [1], DT, kind="ExternalInput")
    out = nc.dram_tensor("out", [NPAD], DT, kind="ExternalOutput")

    rg = [list(range(N_CORES))]

    from contextlib import ExitStack

    with tile.TileContext(nc) as tc, ExitStack() as es:
        sb = es.enter_context(tc.tile_pool(name="sb", bufs=1))
        wk = es.enter_context(tc.tile_pool(name="wk", bufs=2))
        pp = es.enter_context(tc.tile_pool(name="pp", bufs=2, space="PSUM"))
        dr = es.enter_context(tc.tile_pool(name="dr", bufs=1, space="DRAM"))

        # ---- persistent DRAM scratch ----------------------------------
        T0_dram = dr.tile([TBL, 1], DT)
        C0_dram = dr.tile([ZROW + 1, 1], DT)
        C1_dram = dr.tile([ZROW + 1, F2], DT)
        C2_dram = dr.tile([ZROW + 1, 1], DT)
        t0_dram = dr.tile([NPAD], DT)
        ag1_in = dr.tile([NPAD, F2], DT)
        T1_dram = dr.tile([TBL, F2], DT, addr_space="Shared")
        ag2_in = dr.tile([NPAD, 1], DT)
        T2_dram = dr.tile([TBL, 1], DT, addr_space="Shared")

        # ---- prep: constants, indices, degree tables ------------------
        ut = sb.tile([P, P], DT)
        make_upper_triangular(nc, ut[:], val=1.0, diag=True)

        idxg = sb.tile([128, W], IT)
        nc.sync.dma_start(idxg[:], sidx_g[:, :])
        idxk = sb.tile([128, W], IT)
        nc.sync.dma_start(idxk[:], sidx_k[:, :])
        bbt = sb.tile([128, NI], IT)
        nc.sync.dma_start(bbt[:], bnd_beg[:, :])
        bet = sb.tile([128, NI], IT)
        nc.sync.dma_start(bet[:], bnd_end[:, :])

        w0sb = sb.tile([1, F1], DT, padded_shape=[128, F1])
        nc.sync.dma_start(w0sb[:], w0[:, :])
        b0col = sb.tile([F1, 1], DT)
        nc.sync.dma_start(b0col[:], b0[:, None])
        w1sb = sb.tile([F1, F2], DT)
        nc.sync.dma_start(w1sb[:], w1[:, :])
        b1rep = sb.tile([P, NI * F2], DT)
        nc.gpsimd.dma_start(
            out=b1rep[:].rearrange("p (a b) -> p a b", b=F2),
            in_=bass.AP(b1.ap().tensor, 0, [[0, P], [0, NI], [1, F2]]),
        )
        w2rep = sb.tile([P, NI * F2], DT)
        nc.gpsimd.dma_start(
            out=w2rep[:].rearrange("p (a b) -> p a b", b=F2),
            in_=bass.AP(w2.ap().tensor, 0, [[0, P], [0, NI], [1, F2]]),
        )
        b2col = sb.tile([P, 1], DT)
        nc.gpsimd.dma_start(
            out=b2col[:], in_=bass.AP(b2.ap().tensor, 0, [[0, P], [1, 1]])
        )

        ddo = sb.tile([P, NI], DT)
        nc.sync.dma_start(ddo[:], deg_dst_own.ap().rearrange("(p i) -> p i", p=128))
        inv_in = sb.tile([P, NI], DT)
        _rsqrt(nc, sb, inv_in[:], ddo[:], "in")
        dsg_t = sb.tile([P, NI], DT)
        nc.sync.dma_start(dsg_t[:], deg_src_g.ap().rearrange("(p i) -> p i", p=128))
        inv_og = sb.tile([P, NI], DT)
        _rsqrt(nc, sb, inv_og[:], dsg_t[:], "og")
        dsk_t = sb.tile([P, NI], DT)
        nc.sync.dma_start(dsk_t[:], deg_src_k.ap().rearrange("(p i) -> p i", p=128))
        inv_ok = sb.tile([P, NI], DT)
        _rsqrt(nc, sb, inv_ok[:], dsk_t[:], "ok")

        # T0 table: x * inv_sqrt(out-degree), all nodes (table space)
        xt_sb = sb.tile([P, TW], DT)
        nc.sync.dma_start(xt_sb[:], x_t.ap().rearrange("(p i) -> p i", p=128))
        dst_sb = sb.tile([P, TW], DT)
        nc.sync.dma_start(dst_sb[:], deg_src_t.ap().rearrange("(p i) -> p i", p=128))
        inv_t = sb.tile([P, TW], DT)
        _rsqrt(nc, sb, inv_t[:], dst_sb[:], "t")
        t0sb = sb.tile([P, TW], DT)
        nc.vector.tensor_tensor(
            out=t0sb[:], in0=xt_sb[:], in1=inv_t[:], op=mybir.AluOpType.mult
        )
        nc.sync.dma_start(
            T0_dram[:, :].rearrange("(p i) f -> p (i f)", p=128), t0sb[:]
        )

        # zero rows of the cumsum scratch buffers
        zrow = sb.tile([1, F2], DT, padded_shape=[128, F2])
        nc.gpsimd.memset(zrow[:], 0.0)
        nc.sync.dma_start(C0_dram[ZROW : ZROW + 1, :], zrow[:, :1])
        nc.sync.dma_start(C1_dram[ZROW : ZROW + 1, :], zrow[:, :])
        nc.sync.dma_start(C2_dram[ZROW : ZROW + 1, :], zrow[:, :1])

        # ---- helpers --------------------------------------------------
        def edge_gather(dst_tile, idx_tile, table, F):
            # HW honors exactly one offset per partition per indirect DMA:
            # one instruction per bin-packed column (128 edges each).
            for s in range(W):
                nc.gpsimd.indirect_dma_start(
                    out=dst_tile[:, s * F : (s + 1) * F],
                    out_offset=None,
                    in_=table[:, :],
                    in_offset=bass.IndirectOffsetOnAxis(
                        ap=idx_tile[:, s : s + 1], axis=0
                    ),
                )

        def cumsum_to_dram(msg_tile, c_dram, F, tag):
            width = W * F
            cs = sb.tile([P, width], DT, name=f"cs_{tag}", tag=f"cs_{tag}")
            step = 510 if F == F2 else 512
            for o in range(0, width, step):
                wn = min(step, width - o)
                ps = pp.tile([P, 512], DT, space="PSUM", tag="cums")
                nc.tensor.matmul(
                    out=ps[:, :wn],
                    lhsT=ut[:],
                    rhs=msg_tile[:, o : o + wn],
                    start=True,
                    stop=True,
                )
                nc.vector.tensor_copy(cs[:, o : o + wn], ps[:, :wn])
            nc.sync.dma_start(
                c_dram[0:ZROW, :].rearrange("(p i) f -> p (i f)", p=128), cs[:]
            )

        def bnd_diff(c_dram, F, tag):
            """gather end/beg rows of c_dram, return (end-beg) tile [P, NI*F]."""
            et = wk.tile([P, NI * F], DT, name=f"e_{tag}", tag=f"e_{tag}")
            bt = wk.tile([P, NI * F], DT, name=f"b_{tag}", tag=f"b_{tag}")
            for s in range(NI):
                nc.gpsimd.indirect_dma_start(
                    out=et[:, s * F : (s + 1) * F],
                    out_offset=None,
                    in_=c_dram[:, :],
                    in_offset=bass.IndirectOffsetOnAxis(ap=bet[:, s : s + 1], axis=0),
                )
                nc.gpsimd.indirect_dma_start(
                    out=bt[:, s * F : (s + 1) * F],
                    out_offset=None,
                    in_=c_dram[:, :],
                    in_offset=bass.IndirectOffsetOnAxis(ap=bbt[:, s : s + 1], axis=0),
                )
            ag = wk.tile([P, NI * F], DT, name=f"ag_{tag}", tag=f"ag_{tag}")
            nc.vector.tensor_tensor(
                out=ag[:], in0=et[:], in1=bt[:], op=mybir.AluOpType.subtract
            )
            return ag

        # ---- layer 0 (F=1) -------------------------------------------
        m0 = sb.tile([P, W], DT)
        edge_gather(m0, idxg, T0_dram, 1)
        tap("dbg_m0", m0[:])
        cumsum_to_dram(m0, C0_dram, 1, "c0")
        tap("dbg_cs0", C0_dram[0:ZROW, :].rearrange("(p i) f -> p (i f)", p=128))
        agg0 = bnd_diff(C0_dram, 1, "l0")
        tap("dbg_agg0", agg0[:])
        t0n = wk.tile([P, NI], DT)
        nc.vector.tensor_tensor(
            out=t0n[:], in0=agg0[:], in1=inv_in[:], op=mybir.AluOpType.mult
        )
        nc.sync.dma_start(t0_dram[:].rearrange("(p i) -> p i", p=128), t0n[:])
        t0row = sb.tile([1, NPAD], DT)
        nc.sync.dma_start(t0row[:], t0_dram[:])
        tap("dbg_t0row", t0row[:])

        # dense chain: h1 = lrelu(t0 x W0 + b0); msg1 = inv_out*(h1 @ W1)
        t1sl = sb.tile([P, NI * F2], DT)
        for o in range(0, NPAD, 512):
            wn = min(512, NPAD - o)
            ps1 = pp.tile([F1, 512], DT, space="PSUM", tag="ps1")
            nc.tensor.matmul(
                out=ps1[:, :wn],
                lhsT=w0sb[:],
                rhs=t0row[:, o : o + wn],
                start=True,
                stop=True,
            )
            xb = wk.tile([F1, 512], DT, tag="xb")
            nc.scalar.activation(
                xb[:, :wn],
                ps1[:, :wn],
                mybir.ActivationFunctionType.Identity,
                bias=b0col[:],
            )
            x01 = wk.tile([F1, 512], DT, tag="x01")
            nc.vector.tensor_scalar(
                out=x01[:, :wn],
                in0=ps1[:, :wn],
                scalar1=b0col[:],
                scalar2=0.01,
                op0=mybir.AluOpType.add,
                op1=mybir.AluOpType.mult,
            )
            h1c = wk.tile([F1, 512], DT, tag="h1c")
            nc.vector.tensor_tensor(
                out=h1c[:, :wn], in0=xb[:, :wn], in1=x01[:, :wn],
                op=mybir.AluOpType.max,
            )
            for sub in range(0, wn, 128):
                k = (o + sub) // 128
                ps3 = pp.tile([P, F2], DT, space="PSUM", tag="ps3")
                nc.tensor.matmul(
                    out=ps3[:],
                    lhsT=h1c[:, sub : sub + 128],
                    rhs=w1sb[:],
                    start=True,
                    stop=True,
                )
                nc.vector.tensor_scalar_mul(
                    t1sl[:, k * F2 : (k + 1) * F2], ps3[:], inv_ok[:, k : k + 1]
                )
        tap("dbg_t1sl", t1sl[:])
        nc.sync.dma_start(
            ag1_in[:, :].rearrange("(p i) f -> p (i f)", p=128), t1sl[:]
        )
        if timing:
            nc.sync.dma_start(T1_dram[0:NPAD, :], ag1_in[:, :])
        else:
            nc.gpsimd.collective_compute(
                "AllGather",
                mybir.AluOpType.bypass,
                replica_groups=rg,
                ins=[ag1_in.opt()],
                outs=[T1_dram.opt()],
            )
        tap("dbg_T1", T1_dram[:, :])

        # ---- layer 1 (F=10) ------------------------------------------
        m1 = sb.tile([P, W * F2], DT)
        edge_gather(m1, idxk, T1_dram, F2)
        tap("dbg_m1", m1[:])
        cumsum_to_dram(m1, C1_dram, F2, "c1")
        agg1 = bnd_diff(C1_dram, F2, "l1")
        tap("dbg_agg1", agg1[:])
        inv_in_rep = bass.AP(
            inv_in[:].tensor, inv_in[:].offset,
            [inv_in[:].ap[0], [1, NI], [0, F2]],
        )
        mm1 = wk.tile([P, NI * F2], DT)
        nc.vector.tensor_tensor(
            out=mm1[:].rearrange("p (a b) -> p a b", b=F2),
            in0=agg1[:].rearrange("p (a b) -> p a b", b=F2),
            in1=inv_in_rep,
            op=mybir.AluOpType.mult,
        )
        h2a = wk.tile([P, NI * F2], DT)
        nc.vector.tensor_tensor(
            out=h2a[:], in0=mm1[:], in1=b1rep[:], op=mybir.AluOpType.add
        )
        h2 = wk.tile([P, NI * F2], DT)
        nc.vector.tensor_scalar_max(h2[:], h2a[:], 0.0)
        # msg2 = inv_out_g * (h2 @ W2)
        hw2 = wk.tile([P, NI * F2], DT)
        nc.vector.tensor_tensor(
            out=hw2[:], in0=h2[:], in1=w2rep[:], op=mybir.AluOpType.mult
        )
        red = wk.tile([P, NI], DT)
        nc.vector.reduce_sum(
            red[:, :, None],
            hw2[:].rearrange("p (a b) -> p a b", b=F2),
            axis=mybir.AxisListType.X,
        )
        t2sl = sb.tile([P, NI], DT)
        nc.vector.tensor_tensor(
            out=t2sl[:], in0=red[:], in1=inv_og[:], op=mybir.AluOpType.mult
        )
        tap("dbg_t2sl", t2sl[:])
        nc.sync.dma_start(
            ag2_in[:, :].rearrange("(p i) f -> p (i f)", p=128), t2sl[:]
        )
        if timing:
            nc.sync.dma_start(T2_dram[0:NPAD, :], ag2_in[:, :])
        else:
            nc.gpsimd.collective_compute(
                "AllGather",
                mybir.AluOpType.bypass,
                replica_groups=rg,
                ins=[ag2_in.opt()],
                outs=[T2_dram.opt()],
            )
        tap("dbg_T2", T2_dram[:, :])

        # ---- layer 2 (F=1) -------------------------------------------
        m2 = sb.tile([P, W], DT)
        edge_gather(m2, idxg, T2_dram, 1)
        tap("dbg_m2", m2[:])
        cumsum_to_dram(m2, C2_dram, 1, "c2")
        agg2 = bnd_diff(C2_dram, 1, "l2")
        tap("dbg_agg2", agg2[:])
        t2n = wk.tile([P, NI], DT)
        nc.vector.tensor_tensor(
            out=t2n[:], in0=agg2[:], in1=inv_in[:], op=mybir.AluOpType.mult
        )
        h3 = wk.tile([P, NI], DT)
        nc.vector.tensor_scalar(
            out=h3[:],
            in0=t2n[:],
            scalar1=b2col[:],
            scalar2=0.0,
            op0=mybir.AluOpType.add,
            op1=mybir.AluOpType.max,
        )
        nc.sync.dma_start(out.ap().rearrange("(p i) -> p i", p=128), h3[:])

    nc.compile()
    return nc


def build_in_maps(in_feat, edge_index, W0, b0, W1, b1, W2, b2):
    cores, deg_src_t, W = _preprocess(np.asarray(edge_index))
    x = np.asarray(in_feat, np.float32).reshape(-1)
    x_t = np.zeros(TBL, np.float32)
    for c in range(N_CORES):
        x_t[c * NPAD : c * NPAD + NPC] = x[c * NPC : (c + 1) * NPC]
    common = dict(
        x_t=x_t,
        deg_src_t=deg_src_t,
        w0=np.asarray(W0, np.float32).reshape(1, F1),
        b0=np.asarray(b0, np.float32).reshape(F1),
        w1=np.asarray(W1, np.float32).reshape(F1, F2),
        b1=np.asarray(b1, np.float32).reshape(F2),
        w2=np.asarray(W2, np.float32).reshape(F2),
        b2=np.asarray(b2, np.float32).reshape(1),
    )
    in_maps = []
    for c in range(N_CORES):
        d = cores[c]
        in_maps.append(
            dict(
                common,
                deg_dst_own=d["deg_dst_own"],
                deg_src_g=d["deg_src_g"],
                deg_src_k=d["deg_src_k"],
                sidx_g=d["sidx_g"],
                sidx_k=d["sidx_k"],
                bnd_beg=d["bnd_beg"],
                bnd_end=d["bnd_end"],
            )
        )
    return in_maps, W


def assemble(results):
    full = np.zeros((N_NODES, 1), np.float32)
    for c in range(N_CORES):
        full[c * NPC : (c + 1) * NPC, 0] = results[c]["out"][:NPC]
    return full


def kernel(in_feat, edge_index, W0, b0, W1, b1, W2, b2):
    in_maps, W = build_in_maps(in_feat, edge_index, W0, b0, W1, b1, W2, b2)
    nc = _build(W)
    res = run_bass_kernel_spmd(
        nc, in_maps, core_ids=list(range(N_CORES)), trace=False
    )
    return assemble(res.results)


# revision 42
# speedup vs baseline: 1.0734x; 1.0734x over previous
"""GCN (3-layer DGL GraphConv, norm='both') on 8 TRN2 NeuronCores.

Strategy: nodes partitioned across cores by dst range. Per layer:
  - per-edge messages gathered via indirect DMA from a replicated node table
  - segment-sum over dst implemented as a triangular-matmul cumsum along
    partitions + two boundary gathers + subtract (no scatter needed; edges
    are host-packed so no node's run crosses a 128-slot column)
  - tiny dense GEMMs (W0 outer-product / W1 / W2) on PE
  - AllGather replicates each core's message-table slice between layers.
Host does index-only preprocessing (sort/pack/degree counts); all float
math runs on device.
"""

import sys

import numpy as np

try:
    import concourse.bass as bass  # noqa: F401
except Exception:  # pragma: no cover
    sys.path.insert(0, "/opt/trn_rl_repo")

import concourse.bass as bass
import concourse.bacc as bacc
import concourse.tile as tile
from concourse import mybir
from concourse.bass_utils import run_bass_kernel_spmd
from concourse.masks import make_upper_triangular

N_CORES = 8
N_NODES = 50000
NPC = N_NODES // N_CORES  # 6250 nodes per core
NPAD = 6272               # 49 * 128
NI = NPAD // 128          # 49
TBL = N_CORES * NPAD      # 50176 table rows
TW = TBL // 128           # 392
F1 = 100
F2 = 10
P = 128
GHOST_G = 6250                       # core-0 pad row, G layout (zeroed)
GHOST_K = (6250 % 128) * NI + 6250 // 128  # 5242, core-0 pad row, K layout
DT = mybir.dt.float32
IT = mybir.dt.int32


def _kappa(m):
    return (m % 128) * NI + m // 128


def _preprocess(edge_index):
    """Index-only host prep. Returns per-core dict arrays + W."""
    src = edge_index[0].astype(np.int64)
    dst = edge_index[1].astype(np.int64)
    deg_src = np.bincount(src, minlength=N_NODES).astype(np.float32)
    deg_dst = np.bincount(dst, minlength=N_NODES).astype(np.float32)
    order = np.argsort(dst, kind="stable")
    src_s = src[order]
    dst_s = dst[order]
    bounds = np.searchsorted(dst_s, np.arange(0, N_NODES + 1, NPC))

    packs = []
    Wmax = 0
    for c in range(N_CORES):
        lo, hi = bounds[c], bounds[c + 1]
        s_c = src_s[lo:hi]
        d_c = (dst_s[lo:hi] - c * NPC).astype(np.int64)
        lens = np.bincount(d_c, minlength=NPC).astype(np.int64)
        assert lens.max() <= 128, f"node degree {lens.max()} exceeds 128"
        # class-grouped FFD: class r = n%NI -> psum column r; rows q=n//NI
        # are distinct within a class, so a [slots, q] interval-mask matmul
        # per column accumulates segment sums directly in G-layout
        col = np.zeros(NPC, np.int64)
        slot0 = np.zeros(NPC, np.int64)
        ccols = np.zeros(NI, np.int64)
        for r in range(NI):
            nodes = np.arange(r, NPC, NI)
            fill = []
            for n in nodes[np.argsort(-lens[nodes], kind="stable")]:
                L = int(lens[n])
                if L == 0:
                    break
                for i in range(len(fill)):
                    if fill[i] + L <= 128:
                        col[n] = i
                        slot0[n] = fill[i]
                        fill[i] += L
                        break
                else:
                    col[n] = len(fill)
                    slot0[n] = 0
                    fill.append(L)
            ccols[r] = len(fill)
        packs.append((s_c, d_c, lens, col, slot0, ccols))

    CC = np.max([p[5] for p in packs], axis=0)  # shared per-class budgets
    OFF = np.zeros(NI + 1, np.int64)
    np.cumsum(CC, out=OFF[1:])
    W = int(OFF[-1])
    ZERO_POS = 128 * W
    cores = []
    for c in range(N_CORES):
        s_c, d_c, lens, col, slot0, _cc = packs[c]
        col = col + OFF[np.arange(NPC) % NI]  # class-local -> global column
        run_start = np.zeros(NPC + 1, np.int64)
        np.cumsum(lens, out=run_start[1:])
        e_n = d_c
        j_e = np.arange(len(s_c)) - run_start[e_n]
        p_e = slot0[e_n] + j_e
        i_e = col[e_n]
        sc_core = s_c // NPC
        sc_loc = s_c % NPC
        sidx_g = np.full((128, W), GHOST_G, np.int32)
        sidx_k = np.full((128, W), GHOST_K, np.int32)
        sidx_g[p_e, i_e] = (sc_core * NPAD + sc_loc).astype(np.int32)
        sidx_k[p_e, i_e] = (sc_core * NPAD + _kappa(sc_loc)).astype(np.int32)

        mask = np.zeros((128, W * 128), np.float32)
        for n in range(NPC):
            L = int(lens[n])
            if L:
                mask[slot0[n]:slot0[n] + L, col[n] * 128 + n // NI] = 1.0

        bnd_end = np.full(NPAD, ZERO_POS, np.int64)
        bnd_beg = np.full(NPAD, ZERO_POS, np.int64)
        has = lens > 0
        idxs = np.nonzero(has)[0]
        bnd_end[idxs] = (slot0[idxs] + lens[idxs] - 1) * W + col[idxs]
        bnd_beg[idxs] = np.where(
            slot0[idxs] > 0, (slot0[idxs] - 1) * W + col[idxs], ZERO_POS
        )

        dd_own = np.ones(NPAD, np.float32)
        dd_own[:NPC] = deg_dst[c * NPC : (c + 1) * NPC]
        # pad rows get a huge degree so inv_sqrt ~ 0 zeroes their messages
        # (they are the target of ghost-edge gathers in the next layer)
        dsg = np.full(NPAD, 1e30, np.float32)
        dsg[:NPC] = deg_src[c * NPC : (c + 1) * NPC]
        dsk = np.full(NPAD, 1e30, np.float32)
        dsk[_kappa(np.arange(NPC))] = deg_src[c * NPC : (c + 1) * NPC]
        cores.append(
            dict(
                sidx_g=sidx_g,
                sidx_k=sidx_k,
                mask=mask,
                bnd_end=bnd_end.astype(np.int32).reshape(128, NI),
                bnd_beg=bnd_beg.astype(np.int32).reshape(128, NI),
                deg_dst_own=dd_own,
                deg_src_g=dsg,
                deg_src_k=dsk,
            )
        )

    # table-space (G layout) full arrays, same for every core
    deg_src_t = np.ones(TBL, np.float32)
    for c in range(N_CORES):
        deg_src_t[c * NPAD : c * NPAD + NPC] = deg_src[c * NPC : (c + 1) * NPC]
    return cores, deg_src_t, (W, CC, OFF)


def _rsqrt(nc, pool, out, in_ap, tmp_tag):
    """out = 1/sqrt(max(in,1)) elementwise."""
    t1 = pool.tile(list(out.shape), DT, name=f"rs1_{tmp_tag}", tag=f"rs1_{tmp_tag}")
    nc.vector.tensor_scalar_max(t1[:], in_ap, 1.0)
    t2 = pool.tile(list(out.shape), DT, name=f"rs2_{tmp_tag}", tag=f"rs2_{tmp_tag}")
    nc.scalar.sqrt(t2[:], t1[:])
    nc.vector.reciprocal(out, t2[:])


def _build(cfg, dbg=False, timing=False):
    """Build the SPMD Bass program (identical for all cores).

    timing=True replaces AllGathers with local DRAM copies so the module is
    collective-free and TimelineSim-compatible (numerics wrong, timing of
    local work representative)."""
    W, CC, OFF = cfg
    ZROW = 128 * W
    nc = bacc.Bacc(
        "TRN2",
        target_bir_lowering=False,
        debug=False,
        num_devices=1 if timing else N_CORES,
    )
    dbg_specs = {
        "dbg_m0": [128, W],
        "dbg_cs0": [128, W],
        "dbg_agg0": [128, NI],
        "dbg_t0row": [1, NPAD],
        "dbg_t1sl": [128, NI * F2],
        "dbg_T1": [TBL, F2],
        "dbg_m1": [128, W * F2],
        "dbg_agg1": [128, NI * F2],
        "dbg_t2sl": [128, NI],
        "dbg_T2": [TBL, 1],
        "dbg_m2": [128, W],
        "dbg_agg2": [128, NI],
    }
    dbg_t = {}
    if dbg:
        for name, shp in dbg_specs.items():
            dbg_t[name] = nc.dram_tensor(name, shp, DT, kind="ExternalOutput")

    def tap(name, ap):
        if dbg:
            nc.sync.dma_start(dbg_t[name].ap(), ap)
    # ---- I/O -----------------------------------------------------------
    x_t = nc.dram_tensor("x_t", [TBL], DT, kind="ExternalInput")
    deg_src_t = nc.dram_tensor("deg_src_t", [TBL], DT, kind="ExternalInput")
    deg_dst_own = nc.dram_tensor("deg_dst_own", [NPAD], DT, kind="ExternalInput")
    deg_src_g = nc.dram_tensor("deg_src_g", [NPAD], DT, kind="ExternalInput")
    deg_src_k = nc.dram_tensor("deg_src_k", [NPAD], DT, kind="ExternalInput")
    sidx_g = nc.dram_tensor("sidx_g", [128, W], IT, kind="ExternalInput")
    sidx_k = nc.dram_tensor("sidx_k", [128, W], IT, kind="ExternalInput")
    mask_in = nc.dram_tensor("mask", [128, W * 128], DT, kind="ExternalInput")
    bnd_beg = nc.dram_tensor("bnd_beg", [128, NI], IT, kind="ExternalInput")
    bnd_end = nc.dram_tensor("bnd_end", [128, NI], IT, kind="ExternalInput")
    w0 = nc.dram_tensor("w0", [1, F1], DT, kind="ExternalInput")
    b0 = nc.dram_tensor("b0", [F1], DT, kind="ExternalInput")
    w1 = nc.dram_tensor("w1", [F1, F2], DT, kind="ExternalInput")
    b1 = nc.dram_tensor("b1", [F2], DT, kind="ExternalInput")
    w2 = nc.dram_tensor("w2", [F2], DT, kind="ExternalInput")
    b2 = nc.dram_tensor("b2", [1], DT, kind="ExternalInput")
    out = nc.dram_tensor("out", [NPAD], DT, kind="ExternalOutput")

    rg = [list(range(N_CORES))]

    from contextlib import ExitStack

    with tile.TileContext(nc) as tc, ExitStack() as es:
        sb = es.enter_context(tc.tile_pool(name="sb", bufs=1))
        wk = es.enter_context(tc.tile_pool(name="wk", bufs=2))
        pp = es.enter_context(tc.tile_pool(name="pp", bufs=2, space="PSUM"))
        dr = es.enter_context(tc.tile_pool(name="dr", bufs=1, space="DRAM"))

        # ---- persistent DRAM scratch ----------------------------------
        T0_dram = dr.tile([TBL, 1], DT)
        C0_dram = dr.tile([ZROW + 1, 1], DT)
        C1_dram = dr.tile([ZROW + 1, F2], DT)
        C2_dram = dr.tile([ZROW + 1, 1], DT)
        t0_dram = dr.tile([NPAD], DT)
        ag1_in = dr.tile([NPAD, F2], DT)
        T1_dram = dr.tile([TBL, F2], DT, addr_space="Shared")
        ag2_in = dr.tile([NPAD, 1], DT)
        T2_dram = dr.tile([TBL, 1], DT, addr_space="Shared")

        # ---- prep: constants, indices, degree tables ------------------
        ut = sb.tile([P, P], DT)
        make_upper_triangular(nc, ut[:], val=1.0, diag=True)

        idxg = sb.tile([128, W], IT)
        nc.sync.dma_start(idxg[:], sidx_g[:, :])
        idxk = sb.tile([128, W], IT)
        nc.sync.dma_start(idxk[:], sidx_k[:, :])
        bbt = sb.tile([128, NI], IT)
        nc.sync.dma_start(bbt[:], bnd_beg[:, :])
        bet = sb.tile([128, NI], IT)
        nc.sync.dma_start(bet[:], bnd_end[:, :])

        w0sb = sb.tile([1, F1], DT, padded_shape=[128, F1])
        nc.sync.dma_start(w0sb[:], w0[:, :])
        b0col = sb.tile([F1, 1], DT)
        nc.sync.dma_start(b0col[:], b0[:, None])
        w1sb = sb.tile([F1, F2], DT)
        nc.sync.dma_start(w1sb[:], w1[:, :])
        b1rep = sb.tile([P, NI * F2], DT)
        nc.gpsimd.dma_start(
            out=b1rep[:].rearrange("p (a b) -> p a b", b=F2),
            in_=bass.AP(b1.ap().tensor, 0, [[0, P], [0, NI], [1, F2]]),
        )
        w2rep = sb.tile([P, NI * F2], DT)
        nc.gpsimd.dma_start(
            out=w2rep[:].rearrange("p (a b) -> p a b", b=F2),
            in_=bass.AP(w2.ap().tensor, 0, [[0, P], [0, NI], [1, F2]]),
        )
        b2col = sb.tile([P, 1], DT)
        nc.gpsimd.dma_start(
            out=b2col[:], in_=bass.AP(b2.ap().tensor, 0, [[0, P], [1, 1]])
        )

        ddo = sb.tile([P, NI], DT)
        nc.sync.dma_start(ddo[:], deg_dst_own.ap().rearrange("(p i) -> p i", p=128))
        inv_in = sb.tile([P, NI], DT)
        _rsqrt(nc, sb, inv_in[:], ddo[:], "in")
        dsg_t = sb.tile([P, NI], DT)
        nc.sync.dma_start(dsg_t[:], deg_src_g.ap().rearrange("(p i) -> p i", p=128))
        inv_og = sb.tile([P, NI], DT)
        _rsqrt(nc, sb, inv_og[:], dsg_t[:], "og")
        dsk_t = sb.tile([P, NI], DT)
        nc.sync.dma_start(dsk_t[:], deg_src_k.ap().rearrange("(p i) -> p i", p=128))
        inv_ok = sb.tile([P, NI], DT)
        _rsqrt(nc, sb, inv_ok[:], dsk_t[:], "ok")

        # T0 table: x * inv_sqrt(out-degree), all nodes (table space)
        xt_sb = sb.tile([P, TW], DT)
        nc.sync.dma_start(xt_sb[:], x_t.ap().rearrange("(p i) -> p i", p=128))
        dst_sb = sb.tile([P, TW], DT)
        nc.sync.dma_start(dst_sb[:], deg_src_t.ap().rearrange("(p i) -> p i", p=128))
        inv_t = sb.tile([P, TW], DT)
        _rsqrt(nc, sb, inv_t[:], dst_sb[:], "t")
        t0sb = sb.tile([P, TW], DT)
        nc.vector.tensor_tensor(
            out=t0sb[:], in0=xt_sb[:], in1=inv_t[:], op=mybir.AluOpType.mult
        )
        nc.sync.dma_start(
            T0_dram[:, :].rearrange("(p i) f -> p (i f)", p=128), t0sb[:]
        )

        # zero rows of the cumsum scratch buffers
        zrow = sb.tile([1, F2], DT, padded_shape=[128, F2])
        nc.gpsimd.memset(zrow[:], 0.0)
        nc.sync.dma_start(C0_dram[ZROW : ZROW + 1, :], zrow[:, :1])
        nc.sync.dma_start(C1_dram[ZROW : ZROW + 1, :], zrow[:, :])
        nc.sync.dma_start(C2_dram[ZROW : ZROW + 1, :], zrow[:, :1])

        # ---- helpers --------------------------------------------------
        def edge_gather(dst_tile, idx_tile, table, F):
            # HW honors exactly one offset per partition per indirect DMA:
            # one instruction per bin-packed column (128 edges each).
            for s in range(W):
                nc.gpsimd.indirect_dma_start(
                    out=dst_tile[:, s * F : (s + 1) * F],
                    out_offset=None,
                    in_=table[:, :],
                    in_offset=bass.IndirectOffsetOnAxis(
                        ap=idx_tile[:, s : s + 1], axis=0
                    ),
                )

        def cumsum_to_dram(msg_tile, c_dram, F, tag):
            width = W * F
            cs = sb.tile([P, width], DT, name=f"cs_{tag}", tag=f"cs_{tag}")
            step = 510 if F == F2 else 512
            for o in range(0, width, step):
                wn = min(step, width - o)
                ps = pp.tile([P, 512], DT, space="PSUM", tag="cums")
                nc.tensor.matmul(
                    out=ps[:, :wn],
                    lhsT=ut[:],
                    rhs=msg_tile[:, o : o + wn],
                    start=True,
                    stop=True,
                )
                nc.vector.tensor_copy(cs[:, o : o + wn], ps[:, :wn])
            nc.sync.dma_start(
                c_dram[0:ZROW, :].rearrange("(p i) f -> p (i f)", p=128), cs[:]
            )

        def bnd_diff(c_dram, F, tag):
            """gather end/beg rows of c_dram, return (end-beg) tile [P, NI*F]."""
            et = wk.tile([P, NI * F], DT, name=f"e_{tag}", tag=f"e_{tag}")
            bt = wk.tile([P, NI * F], DT, name=f"b_{tag}", tag=f"b_{tag}")
            for s in range(NI):
                nc.gpsimd.indirect_dma_start(
                    out=et[:, s * F : (s + 1) * F],
                    out_offset=None,
                    in_=c_dram[:, :],
                    in_offset=bass.IndirectOffsetOnAxis(ap=bet[:, s : s + 1], axis=0),
                )
                nc.gpsimd.indirect_dma_start(
                    out=bt[:, s * F : (s + 1) * F],
                    out_offset=None,
                    in_=c_dram[:, :],
                    in_offset=bass.IndirectOffsetOnAxis(ap=bbt[:, s : s + 1], axis=0),
                )
            ag = wk.tile([P, NI * F], DT, name=f"ag_{tag}", tag=f"ag_{tag}")
            nc.vector.tensor_tensor(
                out=ag[:], in0=et[:], in1=bt[:], op=mybir.AluOpType.subtract
            )
            return ag

        MCH = 16  # mask-load chunk (columns)

        def agg_mm(msg_tile, F, tag):
            """segment-sum per class via interval-mask matmuls, G-layout."""
            ps = pp.tile([P, 512], DT, space="PSUM", tag="aggps")
            for c0 in range(0, W, MCH):
                cw = min(MCH, W - c0)
                mkt = wk.tile([P, MCH * 128], DT, name=f"mk_{tag}", tag="mk")
                nc.scalar.dma_start(
                    mkt[:, :cw * 128], mask_in[:, c0 * 128:(c0 + cw) * 128]
                )
                for c in range(c0, c0 + cw):
                    r = int(np.searchsorted(OFF, c, side="right")) - 1
                    k = c - int(OFF[r])
                    nc.tensor.matmul(
                        out=ps[:, r * F:(r + 1) * F],
                        lhsT=mkt[:, (c - c0) * 128:(c - c0 + 1) * 128],
                        rhs=msg_tile[:, c * F:(c + 1) * F],
                        start=(k == 0),
                        stop=(k == int(CC[r]) - 1),
                        skip_group_check=True,
                    )
            ag = wk.tile([P, NI * F], DT, name=f"ag_{tag}", tag=f"ag_{tag}")
            nc.vector.tensor_copy(ag[:], ps[:, :NI * F])
            return ag

        # ---- layer 0 (F=1) -------------------------------------------
        m0 = sb.tile([P, W], DT)
        edge_gather(m0, idxg, T0_dram, 1)
        tap("dbg_m0", m0[:])
        agg0 = agg_mm(m0, 1, "l0")
        tap("dbg_agg0", agg0[:])
        t0n = wk.tile([P, NI], DT)
        nc.vector.tensor_tensor(
            out=t0n[:], in0=agg0[:], in1=inv_in[:], op=mybir.AluOpType.mult
        )
        nc.sync.dma_start(t0_dram[:].rearrange("(p i) -> p i", p=128), t0n[:])
        t0row = sb.tile([1, NPAD], DT)
        nc.sync.dma_start(t0row[:], t0_dram[:])
        tap("dbg_t0row", t0row[:])

        # dense chain: h1 = lrelu(t0 x W0 + b0); msg1 = inv_out*(h1 @ W1)
        t1sl = sb.tile([P, NI * F2], DT)
        for o in range(0, NPAD, 512):
            wn = min(512, NPAD - o)
            ps1 = pp.tile([F1, 512], DT, space="PSUM", tag="ps1")
            nc.tensor.matmul(
                out=ps1[:, :wn],
                lhsT=w0sb[:],
                rhs=t0row[:, o : o + wn],
                start=True,
                stop=True,
            )
            xb = wk.tile([F1, 512], DT, tag="xb")
            nc.scalar.activation(
                xb[:, :wn],
                ps1[:, :wn],
                mybir.ActivationFunctionType.Identity,
                bias=b0col[:],
            )
            x01 = wk.tile([F1, 512], DT, tag="x01")
            nc.vector.tensor_scalar(
                out=x01[:, :wn],
                in0=ps1[:, :wn],
                scalar1=b0col[:],
                scalar2=0.01,
                op0=mybir.AluOpType.add,
                op1=mybir.AluOpType.mult,
            )
            h1c = wk.tile([F1, 512], DT, tag="h1c")
            nc.vector.tensor_tensor(
                out=h1c[:, :wn], in0=xb[:, :wn], in1=x01[:, :wn],
                op=mybir.AluOpType.max,
            )
            for sub in range(0, wn, 128):
                k = (o + sub) // 128
                ps3 = pp.tile([P, F2], DT, space="PSUM", tag="ps3")
                nc.tensor.matmul(
                    out=ps3[:],
                    lhsT=h1c[:, sub : sub + 128],
                    rhs=w1sb[:],
                    start=True,
                    stop=True,
                )
                nc.vector.tensor_scalar_mul(
                    t1sl[:, k * F2 : (k + 1) * F2], ps3[:], inv_ok[:, k : k + 1]
                )
        tap("dbg_t1sl", t1sl[:])
        nc.sync.dma_start(
            ag1_in[:, :].rearrange("(p i) f -> p (i f)", p=128), t1sl[:]
        )
        if timing:
            nc.sync.dma_start(T1_dram[0:NPAD, :], ag1_in[:, :])
        else:
            nc.gpsimd.collective_compute(
                "AllGather",
                mybir.AluOpType.bypass,
                replica_groups=rg,
                ins=[ag1_in.opt()],
                outs=[T1_dram.opt()],
            )
        tap("dbg_T1", T1_dram[:, :])

        # ---- layer 1 (F=10) ------------------------------------------
        m1 = sb.tile([P, W * F2], DT)
        edge_gather(m1, idxk, T1_dram, F2)
        tap("dbg_m1", m1[:])
        agg1 = agg_mm(m1, F2, "l1")
        tap("dbg_agg1", agg1[:])
        inv_in_rep = bass.AP(
            inv_in[:].tensor, inv_in[:].offset,
            [inv_in[:].ap[0], [1, NI], [0, F2]],
        )
        mm1 = wk.tile([P, NI * F2], DT)
        nc.vector.tensor_tensor(
            out=mm1[:].rearrange("p (a b) -> p a b", b=F2),
            in0=agg1[:].rearrange("p (a b) -> p a b", b=F2),
            in1=inv_in_rep,
            op=mybir.AluOpType.mult,
        )
        h2a = wk.tile([P, NI * F2], DT)
        nc.vector.tensor_tensor(
            out=h2a[:], in0=mm1[:], in1=b1rep[:], op=mybir.AluOpType.add
        )
        h2 = wk.tile([P, NI * F2], DT)
        nc.vector.tensor_scalar_max(h2[:], h2a[:], 0.0)
        # msg2 = inv_out_g * (h2 @ W2)
        hw2 = wk.tile([P, NI * F2], DT)
        nc.vector.tensor_tensor(
            out=hw2[:], in0=h2[:], in1=w2rep[:], op=mybir.AluOpType.mult
        )
        red = wk.tile([P, NI], DT)
        nc.vector.reduce_sum(
            red[:, :, None],
            hw2[:].rearrange("p (a b) -> p a b", b=F2),
            axis=mybir.AxisListType.X,
        )
        t2sl = sb.tile([P, NI], DT)
        nc.vector.tensor_tensor(
            out=t2sl[:], in0=red[:], in1=inv_og[:], op=mybir.AluOpType.mult
        )
        tap("dbg_t2sl", t2sl[:])
        nc.sync.dma_start(
            ag2_in[:, :].rearrange("(p i) f -> p (i f)", p=128), t2sl[:]
        )
        if timing:
            nc.sync.dma_start(T2_dram[0:NPAD, :], ag2_in[:, :])
        else:
            nc.gpsimd.collective_compute(
                "AllGather",
                mybir.AluOpType.bypass,
                replica_groups=rg,
                ins=[ag2_in.opt()],
                outs=[T2_dram.opt()],
            )
        tap("dbg_T2", T2_dram[:, :])

        # ---- layer 2 (F=1) -------------------------------------------
        m2 = sb.tile([P, W], DT)
        edge_gather(m2, idxg, T2_dram, 1)
        tap("dbg_m2", m2[:])
        agg2 = agg_mm(m2, 1, "l2")
        tap("dbg_agg2", agg2[:])
        t2n = wk.tile([P, NI], DT)
        nc.vector.tensor_tensor(
            out=t2n[:], in0=agg2[:], in1=inv_in[:], op=mybir.AluOpType.mult
        )
        h3 = wk.tile([P, NI], DT)
        nc.vector.tensor_scalar(
            out=h3[:],
            in0=t2n[:],
            scalar1=b2col[:],
            scalar2=0.0,
            op0=mybir.AluOpType.add,
            op1=mybir.AluOpType.max,
        )
        nc.sync.dma_start(out.ap().rearrange("(p i) -> p i", p=128), h3[:])

    nc.compile()
    return nc


def build_in_maps(in_feat, edge_index, W0, b0, W1, b1, W2, b2):
    cores, deg_src_t, cfg = _preprocess(np.asarray(edge_index))
    x = np.asarray(in_feat, np.float32).reshape(-1)
    x_t = np.zeros(TBL, np.float32)
    for c in range(N_CORES):
        x_t[c * NPAD : c * NPAD + NPC] = x[c * NPC : (c + 1) * NPC]
    common = dict(
        x_t=x_t,
        deg_src_t=deg_src_t,
        w0=np.asarray(W0, np.float32).reshape(1, F1),
        b0=np.asarray(b0, np.float32).reshape(F1),
        w1=np.asarray(W1, np.float32).reshape(F1, F2),
        b1=np.asarray(b1, np.float32).reshape(F2),
        w2=np.asarray(W2, np.float32).reshape(F2),
        b2=np.asarray(b2, np.float32).reshape(1),
    )
    in_maps = []
    for c in range(N_CORES):
        d = cores[c]
        in_maps.append(
            dict(
                common,
                deg_dst_own=d["deg_dst_own"],
                deg_src_g=d["deg_src_g"],
                deg_src_k=d["deg_src_k"],
                sidx_g=d["sidx_g"],
                sidx_k=d["sidx_k"],
                mask=d["mask"],
                bnd_beg=d["bnd_beg"],
                bnd_end=d["bnd_end"],
            )
        )
    return in_maps, cfg


def assemble(results):
    full = np.zeros((N_NODES, 1), np.float32)
    for c in range(N_CORES):
        full[c * NPC : (c + 1) * NPC, 0] = results[c]["out"][:NPC]
    return full


def kernel(in_feat, edge_index, W0, b0, W1, b1, W2, b2):
    in_maps, cfg = build_in_maps(in_feat, edge_index, W0, b0, W1, b1, W2, b2)
    nc = _build(cfg)
    res = run_bass_kernel_spmd(
        nc, in_maps, core_ids=list(range(N_CORES)), trace=False
    )
    return assemble(res.results)
